# revision 3
# baseline (speedup 1.0000x reference)
"""Causal self-attention (B=1, T=2048, D=1024, H=8, hd=128) on 8 trn2 cores.

Sharding: tensor-parallel over heads -- one head per core. Each core computes
its head's qkv projection, rms-norm+rotary, causal attention, and the c_proj
partial product for its head's 128 columns; the host sums the 8 partial
[D, T] outputs and transposes back to [1, T, D].

Per-core layout strategy (bf16 matmul operands, fp32 PSUM accumulation,
fp32 softmax statistics; measured end-to-end max-rel error ~4e-3):
  - host passes x.T [D, T] in bf16; qkv computed in NATURAL token layout
    (lhsT = x.T chunk tile [c,128t], rhs = Wqkv.T chunk [c,384]) so that
    rms-norm and rotary are free-axis ops. Per-chunk x tiles + 6 PSUM
    bufs let the qkv accumulation start while x is still loading.
  - rms sums-of-squares taken from PSUM (pre-bf16-rounding) on ACT;
    rotary applied in place on the raw q,k (commutes with the per-token
    rms scale) using only the 32 nonzero freqs (cols 32:64 & 96:128 are
    unrotated); the rms scale (with ATTN_SCALE folded in for q) is then
    fused into the [d,t]-transposes as out = q_tile.T @ diag(r) matmuls.
  - scores s.T[s',t] = k-hat-tile.T @ q-hat.T; exp on ACT straight from
    PSUM (logits bounded by 0.12*128=15.36 so no max subtraction is
    needed), probabilities stored bf16, double-buffered across t-blocks.
    Causality via variable-width matmuls (diagonal tiles shrink) + one
    shared [128,128] triangle mask multiply on the diagonal blocks.
  - denominator via an all-ones [128,128] stationary matmul (the result
    is replicated across partitions for free); y.T = v-hat-tiles.T @ p.T
    accumulated in PSUM; y-hat = y.T * recip(denom) in fp32; out.T[e,t]
    = c_projT-tile.T @ y-hat, drained bf16 and summed on the host in
    float64.
"""

import numpy as np

B, T, D = 1, 2048, 1024
H, HD = 8, 128
SCALE = 0.12
NCORES = 8
NT = T // 128      # 16 token tiles
NCH = D // 128     # 8 contraction chunks
NTJ = 4            # attention t-blocks
TJ = T // NTJ      # 512
EPS = float(np.finfo(np.float32).eps)

_CACHE = {}


def _bcast(ap, n):
    """Broadcast a [..., 1] AP to [..., n] via a step-0 trailing dim."""
    try:
        return ap.to_broadcast(list(ap.shape[:-1]) + [n])
    except Exception:
        import concourse.bass as bass
        return bass.AP(tensor=ap.tensor, offset=ap.offset,
                       ap=list(ap.ap[:-1]) + [[0, n]])


def _bcast_mid(ap, n):
    """Insert a step-0 middle dim: [p, f] -> [p, n, f]."""
    import concourse.bass as bass
    return bass.AP(tensor=ap.tensor, offset=ap.offset,
                   ap=[list(ap.ap[0]), [0, n], list(ap.ap[1])])


def _build_program():
    if "nc" in _CACHE:
        return _CACHE["nc"]

    import concourse.bacc as bacc
    import concourse.tile as tile
    import concourse.mybir as mybir

    f32 = mybir.dt.float32
    f32r = mybir.dt.float32r
    bf16 = mybir.dt.bfloat16
    AF = mybir.ActivationFunctionType
    ALU = mybir.AluOpType

    nc = bacc.Bacc("TRN2", target_bir_lowering=False, debug=False)

    xT_d = nc.dram_tensor("xT", [D, T], bf16, kind="ExternalInput")
    w_d = nc.dram_tensor("wqkvT", [128, NCH, 3 * HD], bf16, kind="ExternalInput")
    ve_d = nc.dram_tensor("veN", [128, NT, HD], bf16, kind="ExternalInput")
    cw_d = nc.dram_tensor("cwT", [HD, D], bf16, kind="ExternalInput")
    lam_d = nc.dram_tensor("lam", [128, 2], f32, kind="ExternalInput")
    cos_d = nc.dram_tensor("cosT", [128, NT, 32], bf16, kind="ExternalInput")
    sin_d = nc.dram_tensor("sinT", [128, NT, 32], bf16, kind="ExternalInput")
    tri_d = nc.dram_tensor("tri", [128, 128], bf16, kind="ExternalInput")
    idn_d = nc.dram_tensor("idn", [128, 128], bf16, kind="ExternalInput")
    out_d = nc.dram_tensor("outT", [128, 8, NTJ, TJ], bf16,
                           kind="ExternalOutput")

    with tile.TileContext(nc) as tc:
        with tc.tile_pool(name="const", bufs=1) as cpool, \
             tc.tile_pool(name="work", bufs=1) as wpool:
            # ---- resident inputs ----
            x_sb = [cpool.tile([128, T], bf16, tag=f"x{c}", name=f"x{c}")
                    for c in range(NCH)]
            w_sb = [cpool.tile([128, 3 * HD], bf16, tag=f"w{c}", name=f"w{c}")
                    for c in range(NCH)]
            ve_sb = cpool.tile([128, NT, HD], bf16)
            cw_sb = cpool.tile([HD, D], bf16)
            lam_sb = cpool.tile([128, 2], f32)
            cos_sb = cpool.tile([128, NT, 32], bf16)
            sin_sb = cpool.tile([128, NT, 32], bf16)
            tri_sb = cpool.tile([128, 128], bf16)
            idn_sb = cpool.tile([128, 128], bf16)
            ones_sb = cpool.tile([128, 128], bf16)
            zeros_sb = cpool.tile([128, 128], bf16)
            zero_sb = cpool.tile([128, 1], f32)
            bq_sb = cpool.tile([128, 1], f32)    # eps/SCALE^2 bias for rq
            bk_sb = cpool.tile([128, 1], f32)    # eps bias for rk

            for ch in range(NCH):
                nc.sync.dma_start(x_sb[ch][:], xT_d[128 * ch:128 * (ch + 1), :])
                nc.sync.dma_start(w_sb[ch][:], w_d[:, ch, :])
            nc.sync.dma_start(ve_sb[:], ve_d[:])
            nc.sync.dma_start(cw_sb[:], cw_d[:])
            nc.sync.dma_start(lam_sb[:], lam_d[:])
            nc.sync.dma_start(cos_sb[:], cos_d[:])
            nc.sync.dma_start(sin_sb[:], sin_d[:])
            nc.sync.dma_start(tri_sb[:], tri_d[:])
            nc.sync.dma_start(idn_sb[:], idn_d[:])
            nc.scalar.add_instruction(mybir.InstLoadActFuncSet(
                name="preload_lnexp", act_func_set_id=6, ins=[], outs=[]))
            nc.vector.memset(ones_sb[:], 1.0)
            nc.gpsimd.memset(zeros_sb[:], 0.0)
            nc.vector.memset(zero_sb[:], 0.0)
            nc.vector.memset(bq_sb[:], EPS / (SCALE * SCALE))
            nc.vector.memset(bk_sb[:], EPS)

            # ---- working buffers ----
            qkv = wpool.tile([128, NT, 3 * HD], bf16)     # natural qkv, 3MB
            rs = wpool.tile([128, NT, 2], f32)           # rms scalars q,k
            qT_sb = wpool.tile([128, T], bf16)            # q-hat.T [d, t]
            kT_sb = wpool.tile([128, NT, 128], bf16)      # k-hat.T [d, si, s']
            pT_bufs = [wpool.tile([128, NT, TJ], bf16, tag=f"pT{i}",
                                  name=f"pT{i}") for i in range(2)]

            # v-hat = lam0*v + lam1*ve needs ve prescaled (DVE, early)
            nc.vector.tensor_scalar_mul(ve_sb[:], ve_sb[:], lam_sb[:, 1:2])

            # ================= phase 1: qkv + rms + rotary + transpose ====
            s2 = SCALE * SCALE
            with tc.tile_pool(name="ps_qkv", bufs=6, space="PSUM") as pq, \
                 tc.tile_pool(name="ps_tr", bufs=2, space="PSUM") as ptr, \
                 tc.tile_pool(name="sq", bufs=2) as sqpool:
                for g in range(4):           # groups of 4 token tiles
                    gs = slice(4 * g, 4 * (g + 1))
                    sq_g = sqpool.tile([128, 4, 2 * HD], f32, tag="sqg")
                    for j in range(4):
                        ti = 4 * g + j
                        ps = pq.tile([128, 3 * HD], f32, tag="qkvp")
                        for c in range(NCH):
                            nc.tensor.matmul(
                                ps[:],
                                x_sb[c][:, 128 * ti:128 * (ti + 1)],
                                w_sb[c][:],
                                start=(c == 0),
                                stop=(c == NCH - 1),
                            )
                        # squares of q,k on ACT (reads PSUM)
                        nc.scalar.activation(sq_g[:, j, :], ps[:, 0:2 * HD], AF.Square,
                                             bias=zero_sb[:])
                        # drain raw qkv tile to SBUF (alternate ACT/DVE)
                        if ti % 2 == 0:
                            nc.scalar.copy(qkv[:, ti, :], ps[:])
                        else:
                            nc.vector.tensor_copy(qkv[:, ti, :], ps[:])
                    # per-group sumsq reduce: [128,4,256]->[128,8,128]->X
                    nc.vector.tensor_reduce(
                        rs[:, gs, :].rearrange("p a b -> p (a b)"),
                        sq_g.rearrange("p a (two d) -> p (a two) d", two=2),
                        axis=mybir.AxisListType.X,
                        op=ALU.add,
                    )
                    # rq = 1/sqrt((ms+eps)/SCALE^2), rk = 1/sqrt(ms+eps)
                    nc.scalar.activation(rs[:, gs, 0], rs[:, gs, 0], AF.Ln,
                                         scale=1.0 / (HD * s2), bias=bq_sb[:])
                    nc.scalar.activation(rs[:, gs, 1], rs[:, gs, 1], AF.Ln,
                                         scale=1.0 / HD, bias=bk_sb[:])
                    nc.scalar.activation(rs[:, gs, :]
                                         .rearrange("p a b -> p (a b)"),
                                         rs[:, gs, :]
                                         .rearrange("p a b -> p (a b)"),
                                         AF.Exp, scale=-0.5, bias=zero_sb[:])

                for g in range(4):        # rotary + diag + transpose per group
                    hs = slice(4 * g, 4 * (g + 1))
                    for base in (0, HD):
                        eng = nc.vector if base == 0 else nc.gpsimd
                        x1 = qkv[:, hs, base + 0:base + 32]
                        x2 = qkv[:, hs, base + 64:base + 96]
                        t1 = wpool.tile([128, 4, 32], bf16, tag=f"rot1{base}",
                                        name=f"t1_{base}")
                        t2 = wpool.tile([128, 4, 32], bf16, tag=f"rot2{base}",
                                        name=f"t2_{base}")
                        t3 = wpool.tile([128, 4, 32], bf16, tag=f"rot3{base}",
                                        name=f"t3_{base}")
                        t4 = wpool.tile([128, 4, 32], bf16, tag=f"rot4{base}",
                                        name=f"t4_{base}")
                        eng.tensor_mul(t1[:], x1, cos_sb[:, hs, :])
                        eng.tensor_mul(t2[:], x2, sin_sb[:, hs, :])
                        eng.tensor_mul(t3[:], x2, cos_sb[:, hs, :])
                        eng.tensor_mul(t4[:], x1, sin_sb[:, hs, :])
                        eng.tensor_add(x1, t1[:], t2[:])
                        eng.tensor_sub(x2, t3[:], t4[:])
                    if True:
                        # diag(rq), diag(rk) for fused transpose+normalize
                        gs = slice(4 * g, 4 * (g + 1))
                        dq_g = wpool.tile([128, 4, 128], bf16, tag=f"dq{g % 2}",
                                          name=f"dq_{g}")
                        dk_g = wpool.tile([128, 4, 128], bf16, tag=f"dk{g % 2}",
                                          name=f"dk_{g}")
                        idn_b = _bcast_mid(idn_sb[:], 4)
                        nc.gpsimd.tensor_tensor(
                            dq_g[:], idn_b, _bcast(rs[:, gs, 0:1], HD),
                            op=ALU.mult)
                        nc.gpsimd.tensor_tensor(
                            dk_g[:], idn_b, _bcast(rs[:, gs, 1:2], HD),
                            op=ALU.mult)
                        # transpose+normalize: out = qtile.T @ diag(r) (PE)
                        for base, isq in ((0, True), (HD, False)):
                            tp = ptr.tile([128, 4, 128], f32, tag="trp")
                            dmat = dq_g if isq else dk_g
                            for j in range(4):
                                ti = 4 * g + j
                                nc.tensor.matmul(
                                    tp[:, j, :],
                                    qkv[:, ti, base:base + HD],
                                    dmat[:, j, :], start=True, stop=True)
                            dview = qT_sb[:, 512 * g:512 * (g + 1)] if isq \
                                else kT_sb[:, 4 * g:4 * (g + 1), :] \
                                .rearrange("p a b -> p (a b)")
                            tpf = tp[:].rearrange("p a b -> p (a b)")
                            if g % 2 == 0:
                                nc.vector.tensor_copy(dview, tpf)
                            else:
                                nc.scalar.copy(dview, tpf)
                        # v-hat in place (DVE)
                        nc.vector.scalar_tensor_tensor(
                            qkv[:, gs, 2 * HD:], qkv[:, gs, 2 * HD:],
                            lam_sb[:, 0:1], ve_sb[:, gs, :],
                            op0=ALU.mult, op1=ALU.add)

            # ================= phase 2: attention + c_proj ================
            with tc.tile_pool(name="ps_sc", bufs=1, space="PSUM") as psc, \
                 tc.tile_pool(name="ps_dg", bufs=2, space="PSUM") as pdg, \
                 tc.tile_pool(name="ps_dn", bufs=1, space="PSUM") as pdn, \
                 tc.tile_pool(name="ps_y", bufs=1, space="PSUM") as py, \
                 tc.tile_pool(name="ps_o", bufs=2, space="PSUM") as po, \
                 tc.tile_pool(name="fin", bufs=4) as fin:
                for tj in range(NTJ):
                    pT_sb = pT_bufs[tj % 2]
                    t0 = TJ * tj
                    n_full = 4 * tj
                    n_act = 4 * (tj + 1)
                    # full score tiles, pairs share one 2-bank psum + one exp
                    for pi in range(n_full // 2):
                        sc = psc.tile([128, 2, TJ], f32, tag="scp")
                        for h2 in range(2):
                            si = 2 * pi + h2
                            nc.tensor.matmul(
                                sc[:, h2, :], kT_sb[:, si, :],
                                qT_sb[:, t0:t0 + TJ],
                                start=True, stop=True)
                        nc.scalar.activation(
                            pT_sb[:, 2 * pi:2 * pi + 2, :]
                            .rearrange("p a b -> p (a b)"),
                            sc[:].rearrange("p a b -> p (a b)"), AF.Exp,
                            bias=zero_sb[:])
                    # diagonal tiles (variable width) + triangle mask
                    for o in range(4):
                        si = 4 * tj + o
                        w = TJ - 128 * o
                        sc = pdg.tile([128, TJ], f32, tag="scd")
                        nc.tensor.matmul(
                            sc[:, 0:w], kT_sb[:, si, :],
                            qT_sb[:, t0 + 128 * o:t0 + TJ],
                            start=True, stop=True)
                        nc.scalar.activation(
                            pT_sb[:, si, 128 * o:TJ], sc[:, 0:w], AF.Exp,
                            bias=zero_sb[:])
                        nc.vector.tensor_mul(
                            pT_sb[:, si, 128 * o:128 * o + 128],
                            pT_sb[:, si, 128 * o:128 * o + 128], tri_sb[:])
                    # y.T accumulation (denominator now via cheap
                    # free-size-1 matmuls below)
                    yT = py.tile([128, TJ], f32, tag="yT")
                    for si in range(n_act):
                        o = si - 4 * tj
                        off = 128 * o if o > 0 else 0
                        nc.tensor.matmul(yT[:, off:TJ],
                                         qkv[:, si, 2 * HD:],
                                         pT_sb[:, si, off:TJ],
                                         start=(si == 0),
                                         stop=(si == n_act - 1))
                    # dn[t-tile] in [t-part, 4]: one PSUM group over the
                    # bank, opened/closed by zero-contribution matmuls so
                    # per-column accumulations can interleave safely
                    dnp = pdn.tile([128, 4], f32, tag="dn")
                    nc.tensor.matmul(dnp[:], zeros_sb[:], ones_sb[:, 0:4],
                                     start=True, stop=False)
                    for o in range(4):
                        for si in range(4 * tj + o + 1):
                            nc.tensor.matmul(
                                dnp[:, o:o + 1],
                                pT_sb[:, si, 128 * o:128 * (o + 1)],
                                ones_sb[:, 0:1], start=False, stop=False)
                    nc.tensor.matmul(dnp[:], zeros_sb[:], ones_sb[:, 0:4],
                                     start=False, stop=True)
                    # 1/dn -> [1,512] row -> replicate across partitions
                    rdn4 = fin.tile([128, 4], bf16, tag="rdn")
                    with nc.allow_low_precision(reason="bf16 1/dn scale, "
                                                "~0.2% uniform per token"):
                        nc.vector.reciprocal(rdn4[:], dnp[:])
                    rdnT = pdg.tile([128, TJ], f32, tag="scd")
                    for o in range(4):
                        nc.tensor.matmul(rdnT[0:1, 128 * o:128 * (o + 1)],
                                         rdn4[:, o:o + 1], idn_sb[:],
                                         start=True, stop=True)
                    rdnT_s = fin.tile([1, TJ], bf16, tag="rdnTs")
                    nc.vector.tensor_copy(rdnT_s[:], rdnT[0:1, :])
                    rep = pdg.tile([128, TJ], f32, tag="scd")
                    nc.tensor.matmul(rep[:], ones_sb[0:1, :], rdnT_s[:],
                                     start=True, stop=True)
                    # vector ops may read only ONE input from PSUM: stage
                    # the replicated 1/dn in SBUF before the yT multiply
                    rep_sb = fin.tile([128, TJ], bf16, tag="repsb")
                    nc.vector.tensor_copy(rep_sb[:], rep[:])
                    yh = fin.tile([128, TJ], bf16, tag="yh")
                    nc.vector.tensor_mul(yh[:], yT[:], rep_sb[:])
                    # c_proj: out.T[e, t] partial for this head
                    act_share = 2 if tj == NTJ - 1 else 4
                    for ep in range(4):
                        st = fin.tile([128, 2, TJ], bf16, tag="st")
                        for h2 in range(2):
                            e = 2 * ep + h2
                            pot = po.tile([128, TJ], f32, tag="pot")
                            nc.tensor.matmul(pot[:],
                                             cw_sb[:, 128 * e:128 * (e + 1)],
                                             yh[:],
                                             start=True, stop=True)
                            if e % act_share == act_share - 1:
                                nc.scalar.copy(st[:, h2, :], pot[:])
                            else:
                                nc.vector.tensor_copy(st[:, h2, :], pot[:])
                        nc.sync.dma_start(
                            out_d[:, 2 * ep:2 * ep + 2, tj, :], st[:])

    nc.compile()
    _CACHE["nc"] = nc
    return nc


def _host_inputs(x, ve, qkv_w, lambdas, c_proj_w):
    """Build the 8 per-core input maps (layout transforms only)."""
    import ml_dtypes
    bf = ml_dtypes.bfloat16
    x = np.asarray(x, np.float32)
    ve = np.asarray(ve, np.float32)
    qkv_w = np.asarray(qkv_w, np.float32)
    lambdas = np.asarray(lambdas, np.float32)
    c_proj_w = np.asarray(c_proj_w, np.float32)

    xT = np.ascontiguousarray(x[0].T.astype(bf))           # [D, T]
    lam = np.ascontiguousarray(np.tile(lambdas.reshape(1, 2), (128, 1)))

    freq = (1.0 / 1024.0) ** np.linspace(0.0, 1.0, HD // 4, dtype=np.float32)
    theta = np.arange(T, dtype=np.float32)[:, None] * freq[None, :]  # [T, 32]
    cosT = np.ascontiguousarray(
        np.cos(theta).astype(bf).reshape(NT, 128, 32).transpose(1, 0, 2))
    sinT = np.ascontiguousarray(
        np.sin(theta).astype(bf).reshape(NT, 128, 32).transpose(1, 0, 2))
    tri = (np.arange(128)[None, :] >= np.arange(128)[:, None]).astype(bf)
    idn = np.eye(128, dtype=np.float32).astype(bf)

    in_maps = []
    for h in range(NCORES):
        sl = slice(128 * h, 128 * (h + 1))
        # Wqkv.T chunks: [c_in 128, chunk 8, (q|k|v) 384]
        wh = qkv_w[:, sl, :]                               # [3, 128, 1024]
        wt = wh.transpose(2, 0, 1).reshape(NCH, 128, 3, HD)  # [c, ci, 3, d]
        wt = np.ascontiguousarray(
            wt.transpose(1, 0, 2, 3).reshape(128, NCH, 3 * HD).astype(bf))
        veh = np.ascontiguousarray(
            ve[0, :, sl].reshape(NT, 128, HD).transpose(1, 0, 2).astype(bf))
        cwh = np.ascontiguousarray(c_proj_w[:, sl].T.astype(bf))  # [128, 1024]
        in_maps.append({
            "xT": xT, "wqkvT": wt, "veN": veh, "cwT": cwh, "lam": lam,
            "cosT": cosT, "sinT": sinT, "tri": tri, "idn": idn,
        })
    return in_maps


def run(x, ve, qkv_w, lambdas, c_proj_w, trace=False):
    from concourse.bass_utils import run_bass_kernel_spmd

    nc = _build_program()
    in_maps = _host_inputs(x, ve, qkv_w, lambdas, c_proj_w)
    res = run_bass_kernel_spmd(
        nc, in_maps, core_ids=list(range(NCORES)), trace=trace)
    acc = np.zeros((128, 8, NTJ, TJ), np.float64)
    for r in res.results:
        acc += r["outT"].astype(np.float64)
    # [p, e, tj, t] -> [e*128+p, tj*512+t] = [D, T]
    out = acc.transpose(1, 0, 2, 3).reshape(D, T)
    out = out.astype(np.float32).T.reshape(B, T, D)
    return out, res


def kernel(x, ve, qkv_w, lambdas, c_proj_w):
    out, _ = run(x, ve, qkv_w, lambdas, c_proj_w, trace=False)
    return out

